# revision 12
# baseline (speedup 1.0000x reference)
"""Bass/Trainium2 kernel for attention-energy softmax:
  proj = enc @ W.T + b        [S,B,D]
  energies[b,s] = hidden[b] . proj[s,b]
  out = softmax(energies, axis=s)[:, None, :]

Algebraic fusion: energies[b,s] = (hidden[b] @ W) . enc[s,b] + hidden[b].b
The bias term is constant per b and cancels in softmax, so it is dropped.
v = hidden @ W is a tiny [B,D]x[D,D] matmul; the kernel then streams the
512MB encoder_outputs once (memory bound), data-parallel over B on 8 cores.
"""

import numpy as np

import concourse.bass as bass
import concourse.mybir as mybir
from concourse import bacc
from concourse.bass_isa import ReduceOp
from concourse.bass_utils import run_bass_kernel_spmd
from concourse.masks import make_identity
from concourse.tile import TileContext

S, B, D = 2048, 64, 1024
NCORES = 8
BL = B // NCORES  # 8 local batches per core
P = 128
T = S // P  # 16 seq tiles
EC = D // P  # 8 contraction chunks
F32 = mybir.dt.float32

TRACE = False  # test.py sets True to profile

_CACHE = {}


def build_kernel() -> bass.Bass:
    nc = bacc.Bacc(None, target_bir_lowering=False)
    enc = nc.dram_tensor("enc", [S, BL, D], F32, kind="ExternalInput")
    # wx = [W | hidden_local.T]: one DMA so the first matmul has a single wait
    wx = nc.dram_tensor("wx", [D, D + BL], F32, kind="ExternalInput")
    out = nc.dram_tensor("out", [BL, S], F32, kind="ExternalOutput")
    v_dram = nc.dram_tensor("vscratch", [BL, D], F32, kind="Internal")

    with TileContext(nc) as tc:
        with (
            tc.tile_pool(name="consts", bufs=1) as consts,
            tc.tile_pool(name="work", bufs=3) as work,
            tc.tile_pool(name="small", bufs=2) as small,
            tc.tile_pool(name="psum", bufs=2, space="PSUM") as psum,
            tc.tile_pool(name="psum_tr", bufs=2, space="PSUM") as psum_tr,
        ):
            ident = consts.tile([P, P], F32)
            make_identity(nc, ident)
            # Dummy PE touch of ident so later transposes don't need a
            # gpsimd wait (matmuls allow only one sync wait).
            warm_ps = psum_tr.tile([P, P], F32, tag="tr")
            nc.tensor.transpose(warm_ps, ident, ident)

            # ---- v = hidden_local @ W  -> [BL, D] ----
            wx_sb = consts.tile([P, EC, D + BL], F32)
            nc.sync.dma_start(
                out=wx_sb, in_=wx[:, :].rearrange("(c p) d -> p c d", p=P)
            )

            v_sb = consts.tile([BL, D], F32)
            for h in range(2):
                v_ps = psum.tile([BL, 512], F32)
                for c in range(EC):
                    nc.tensor.matmul(
                        v_ps,
                        wx_sb[:, c, D : D + BL],
                        wx_sb[:, c, h * 512 : (h + 1) * 512],
                        start=(c == 0),
                        stop=(c == EC - 1),
                    )
                nc.scalar.copy(out=v_sb[:, h * 512 : (h + 1) * 512], in_=v_ps)

            # ---- broadcast v rows to all 128 partitions: Vb[p, b, d] = v[b, d] ----
            # DRAM roundtrip: write v, read it back with a stride-0 partition loop.
            nc.sync.dma_start(out=v_dram[:, :], in_=v_sb)
            vb = consts.tile([P, BL, D], F32)
            v_bcast_ap = bass.AP(
                tensor=v_dram,
                offset=0,
                ap=[[0, P], [D, BL], [1, D]],
            )
            nc.sync.dma_start(out=vb, in_=v_bcast_ap)

            # ---- energies: Eall[p, b, t] = sum_d enc[t*128+p, b, d] * v[b, d] ----
            e_all = consts.tile([P, BL, T], F32)
            dummy = consts.tile([P, 1], F32)
            for t in range(T):
                enc_t = work.tile([P, BL, D], F32)
                nc.sync.dma_start(out=enc_t, in_=enc[t * P : (t + 1) * P, :, :])
                for b in range(BL):
                    # fused multiply + free-dim sum in one DVE pass:
                    # out = (in0 * 1.0) * in1, accum = sum(out)
                    nc.vector.scalar_tensor_tensor(
                        out=dummy.broadcast_to((P, D)),
                        in0=enc_t[:, b, :],
                        scalar=1.0,
                        in1=vb[:, b, :],
                        op0=mybir.AluOpType.mult,
                        op1=mybir.AluOpType.mult,
                        accum_out=e_all[:, b, t : t + 1],
                    )

            # ---- softmax over (p, t) per b ----
            m8 = small.tile([P, BL], F32)
            nc.vector.tensor_reduce(
                out=m8, in_=e_all, axis=mybir.AxisListType.X, op=mybir.AluOpType.max
            )
            nc.gpsimd.partition_all_reduce(m8, m8, P, ReduceOp.max)
            neg_g = small.tile([P, BL], F32)
            nc.vector.tensor_scalar_mul(neg_g, m8, -1.0)

            s8 = small.tile([P, BL], F32)
            for b in range(BL):
                nc.scalar.activation(
                    out=e_all[:, b, :],
                    in_=e_all[:, b, :],
                    func=mybir.ActivationFunctionType.Exp,
                    bias=neg_g[:, b : b + 1],
                    accum_out=s8[:, b : b + 1],
                )
            nc.gpsimd.partition_all_reduce(s8, s8, P, ReduceOp.add)
            recip = small.tile([P, BL], F32)
            nc.vector.reciprocal(recip, s8)

            # ---- scale, transpose to [t, b, p] for a contiguous store ----
            out_t = consts.tile([T, BL, P], F32)
            for b in range(BL):
                scl = small.tile([P, T], F32)
                nc.vector.tensor_scalar_mul(scl, e_all[:, b, :], recip[:, b : b + 1])
                tr = psum_tr.tile([T, P], F32, tag="tr")
                nc.tensor.transpose(tr, scl, ident)
                nc.vector.tensor_copy(out=out_t[:, b, :], in_=tr)
            nc.sync.dma_start(
                out=out[:, :].rearrange("b (t p) -> t b p", p=P), in_=out_t
            )

    nc.compile()
    return nc


def kernel(hidden, encoder_outputs, W_attn, b_attn):
    hidden = np.asarray(hidden, dtype=np.float32)
    encoder_outputs = np.asarray(encoder_outputs, dtype=np.float32)
    W_attn = np.asarray(W_attn, dtype=np.float32)

    in_maps = []
    for c in range(NCORES):
        bs = slice(c * BL, (c + 1) * BL)
        in_maps.append(
            {
                "enc": np.ascontiguousarray(encoder_outputs[:, bs, :]),
                "wx": np.ascontiguousarray(
                    np.concatenate([W_attn, hidden[0, bs, :].T], axis=1)
                ),
            }
        )

    if "nc" not in _CACHE:
        _CACHE["nc"] = build_kernel()
    nc = _CACHE["nc"]

    res = run_bass_kernel_spmd(nc, in_maps, core_ids=list(range(NCORES)), trace=TRACE)
    if TRACE:
        _CACHE["last_result"] = res
    out = np.concatenate([r["out"] for r in res.results], axis=0)  # [B, S]
    return out[:, None, :]


# revision 28
# speedup vs baseline: 1.2881x; 1.2881x over previous
"""Bass/Trainium2 kernel for attention-energy softmax:
  proj = enc @ W.T + b        [S,B,D]
  energies[b,s] = hidden[b] . proj[s,b]
  out = softmax(energies, axis=s)[:, None, :]

Algebraic fusion: energies[b,s] = (hidden[b] @ W) . enc[s,b] + hidden[b].b
The bias term is constant per b and cancels in softmax, so it is dropped.
v = hidden @ W is a tiny [B,D]x[D,D] matmul; the kernel then streams the
512MB encoder_outputs once (memory bound), data-parallel over B on 8 cores.

Per-core pipeline:
  1. W|hidden.T arrives as bf16 hi+lo pairs (exact to ~2^-17) in 8 chunked
     DMAs; 3-pass bf16 v matmuls (hi*hi + hi*lo + lo*hi) overlap the load,
     on a pre-warmed PE.
  2. v is re-split hi/lo into bf16 and broadcast to all 128 partitions with
     selector matmuls accumulating hi+lo in PSUM (no HBM traffic).
  3. 15x 4MB + 2x 2MB fully-contiguous DMAs of enc; each followed by fused
     multiply+reduce (scalar_tensor_tensor with accum) ops on DVE.
  4. Softmax in two overlapped groups with on-chip cross-partition max/sum
     via PE transpose / ones-matmul and diagonal-matmul broadcasts.
"""

import numpy as np

import concourse.bass as bass
import concourse.mybir as mybir
from concourse import bacc
from concourse.masks import make_identity
from concourse.bass_utils import run_bass_kernel_spmd
from concourse.tile import TileContext

S, B, D = 2048, 64, 1024
NCORES = 8
BL = B // NCORES  # 8 local batches per core
P = 128
T = S // P  # 16 seq tiles
EC = D // P  # 8 contraction chunks
F32 = mybir.dt.float32
BF16 = mybir.dt.bfloat16

TRACE = False  # test.py sets True to profile

_CACHE = {}


def _stats_exp(nc, small, pstat, ident, ones8, e_all, g0, gw):
    """Cross-partition max and exp (with accumulated sums) for b in
    [g0, g0+gw). Returns the per-partition sums tile."""
    m8g = small.tile([P, gw], F32, tag=f"m8{g0}")
    nc.vector.tensor_reduce(
        out=m8g,
        in_=e_all[:, g0 : g0 + gw, :],
        axis=mybir.AxisListType.X,
        op=mybir.AluOpType.max,
    )
    trm = pstat.tile([gw, P], F32, tag="stat")
    nc.tensor.transpose(trm, m8g, ident)
    mt = small.tile([gw, P], F32, tag=f"mt{g0}")
    nc.vector.tensor_copy(out=mt, in_=trm)
    gmax = small.tile([gw, 1], F32, tag=f"gmax{g0}")
    nc.vector.tensor_reduce(
        out=gmax, in_=mt, axis=mybir.AxisListType.X, op=mybir.AluOpType.max
    )
    gneg = small.tile([gw, 1], F32, tag=f"gneg{g0}")
    nc.vector.tensor_scalar_mul(gneg, gmax, -1.0)
    diag = small.tile([gw, gw], F32, tag=f"diag{g0}")
    nc.vector.tensor_scalar_mul(diag, ident[0:gw, 0:gw], gneg)
    ngps = pstat.tile([P, gw], F32, tag="stat")
    nc.tensor.matmul(ngps, ones8[0:gw, :], diag, start=True, stop=True)
    negg = small.tile([P, gw], F32, tag=f"negg{g0}")
    nc.vector.tensor_copy(out=negg, in_=ngps)
    s8g = small.tile([P, gw], F32, tag=f"s8{g0}")
    for j in range(gw):
        b = g0 + j
        nc.scalar.activation(
            out=e_all[:, b, :],
            in_=e_all[:, b, :],
            func=mybir.ActivationFunctionType.Exp,
            bias=negg[:, j : j + 1],
            accum_out=s8g[:, j : j + 1],
        )
    return s8g


def _recip_bcast(nc, small, pstat, ones_col, ones_row, s8g, gw):
    """1/sum per b, broadcast to all partitions via K=1 ones-matmul."""
    smps = pstat.tile([1, gw], F32, tag="stat")
    nc.tensor.matmul(smps, ones_col, s8g, start=True, stop=True)
    srow = small.tile([1, gw], F32, tag="srow")
    nc.vector.tensor_copy(out=srow, in_=smps)
    rrow = small.tile([1, gw], F32, tag="rrow")
    nc.vector.reciprocal(rrow, srow)
    rps = pstat.tile([P, gw], F32, tag="stat")
    nc.tensor.matmul(rps, ones_row, rrow, start=True, stop=True)
    recipg = small.tile([P, gw], F32, tag="recip")
    nc.vector.tensor_copy(out=recipg, in_=rps)
    return recipg


def build_kernel() -> bass.Bass:
    nc = bacc.Bacc(None, target_bir_lowering=False)
    enc = nc.dram_tensor("enc", [S, BL, D], F32, kind="ExternalInput")
    # wxs = [W_hi | hT_hi | W_lo | hT_lo] as bf16 (hi + lo halves of fp32)
    wxs = nc.dram_tensor("wxs", [D, 2 * (D + BL)], BF16, kind="ExternalInput")
    out = nc.dram_tensor("out", [BL, S], F32, kind="ExternalOutput")
    DB = D + BL

    with TileContext(nc) as tc:
        with (
            tc.tile_pool(name="consts", bufs=1) as consts,
            tc.tile_pool(name="work", bufs=3) as work,
            tc.tile_pool(name="small", bufs=2) as small,
            tc.tile_pool(name="mm", bufs=2, space="PSUM") as mmp,
            tc.tile_pool(name="ptr", bufs=2, space="PSUM") as ptr,
            tc.tile_pool(name="pstat", bufs=2, space="PSUM") as pstat,
        ):
            ident = consts.tile([P, P], F32)
            make_identity(nc, ident)
            # Warm the PE p-state (needs ~3us of continuous work to reach
            # 2.4GHz) while the weight DMAs are in flight, so the v matmuls
            # run at full clock.
            warm_ps = pstat.tile([P, P], F32, tag="warm")
            for _ in range(8):
                nc.tensor.matmul(warm_ps, ident, ident, start=True, stop=True)

            # ---- chunked load of [W|hT] hi+lo; v matmuls overlap the DMA ----
            wx_r = wxs[:, :].rearrange("(c p) d -> p c d", p=P)
            wx_sb = []
            for c in range(EC):
                wt = consts.tile([P, 1, 2 * DB], BF16, tag=f"wx{c}")
                nc.sync.dma_start(out=wt, in_=wx_r[:, c : c + 1, :])
                wx_sb.append(wt)

            def wchunk(c, lo):
                return wx_sb[c][:, 0, lo * DB : lo * DB + D]

            def hchunk(c, lo):
                return wx_sb[c][:, 0, lo * DB + D : lo * DB + DB]

            # selector tiles: sel[k, b, m] = 1 if k == b else 0
            ones8 = consts.tile([BL, P], F32)
            nc.vector.memset(ones8, 1.0)
            sel = consts.tile([BL, BL, P], BF16)
            for b in range(BL):
                nc.vector.tensor_scalar_mul(
                    sel[:, b, :], ones8, ident[0:BL, b : b + 1]
                )

            # v = hidden_local @ W -> [BL, D], 3 bf16 passes (hh + hl + lh),
            # hi/lo re-split per 512-column half so the DVE casts overlap the
            # second half's matmuls
            v_sb = consts.tile([BL, D], F32)
            v_hi = consts.tile([BL, D], BF16)
            v_lo = consts.tile([BL, D], BF16)
            for h in range(2):
                v_ps = mmp.tile([BL, 512], F32, tag="mm")
                first = True
                cols = slice(h * 512, (h + 1) * 512)
                for c in range(EC):
                    for hl, wl in ((0, 0), (0, 1), (1, 0)):
                        nc.tensor.matmul(
                            v_ps,
                            hchunk(c, hl),
                            wchunk(c, wl)[:, cols],
                            start=first,
                            stop=(c == EC - 1 and (hl, wl) == (1, 0)),
                        )
                        first = False
                nc.scalar.copy(out=v_sb[:, cols], in_=v_ps)
                nc.vector.tensor_copy(out=v_hi[:, cols], in_=v_sb[:, cols])
                nc.vector.tensor_sub(v_lo[:, cols], v_sb[:, cols], v_hi[:, cols])

            # ---- broadcast v to all partitions: vb[p, b, d] = v[b, d] ----
            vb = consts.tile([P, BL, D], F32)
            for b in range(BL):
                for h in range(2):
                    bc_ps = mmp.tile([P, 512], F32, tag="mm")
                    nc.tensor.matmul(
                        bc_ps,
                        sel[:, b, :],
                        v_hi[:, h * 512 : (h + 1) * 512],
                        start=True,
                        stop=False,
                    )
                    nc.tensor.matmul(
                        bc_ps,
                        sel[:, b, :],
                        v_lo[:, h * 512 : (h + 1) * 512],
                        start=False,
                        stop=True,
                    )
                    # b=0 copies on the (still idle) vector engine so the
                    # first multiply can start ASAP; the rest stream on ACT
                    eng = nc.vector.tensor_copy if b == 0 else nc.scalar.copy
                    eng(out=vb[:, b, h * 512 : (h + 1) * 512], in_=bc_ps)

            # ---- energies: e_all[p, b, t] = sum_d enc[t*128+p, b, d]*v[b, d] ----
            e_all = consts.tile([P, BL, T], F32)
            dummy = consts.tile([P, 1], F32)
            ones_col = consts.tile([P, 1], F32)
            nc.vector.memset(ones_col, 1.0)
            ones_row = consts.tile([1, P], F32)
            nc.vector.memset(ones_row, 1.0)
            out_r = out[:, :].rearrange("b (t p) -> t b p", p=P)
            out_t = consts.tile([T, BL, P], F32)
            GW = BL // 2  # softmax group width

            def stt(src, j, b, t):
                # fused multiply + free-dim sum in one DVE pass:
                # out = (in0 * 1.0) * in1, accum = sum(out)
                nc.vector.scalar_tensor_tensor(
                    out=dummy.broadcast_to((P, D)),
                    in0=src[:, j, :],
                    scalar=1.0,
                    in1=vb[:, b, :],
                    op0=mybir.AluOpType.mult,
                    op1=mybir.AluOpType.mult,
                    accum_out=e_all[:, b, t : t + 1],
                )

            for t in range(T - 1):
                enc_t = work.tile([P, BL, D], F32, tag="enc_t")
                nc.sync.dma_start(out=enc_t, in_=enc[t * P : (t + 1) * P, :, :])
                for b in range(BL):
                    stt(enc_t, b, b, t)
            # last seq tile split by b-halves so group-0 softmax overlaps
            # the remaining multiply stream
            t = T - 1
            last_halves = []
            for gh in range(2):
                eh = work.tile([P, GW, D], F32, tag="enc_t")
                nc.sync.dma_start(
                    out=eh, in_=enc[t * P : (t + 1) * P, gh * GW : (gh + 1) * GW, :]
                )
                last_halves.append(eh)
            for gh in range(2):
                for j in range(GW):
                    stt(last_halves[gh], j, gh * GW + j, t)
                if gh == 0:
                    s8_0 = _stats_exp(nc, small, pstat, ident, ones8, e_all, 0, GW)
            s8_1 = _stats_exp(nc, small, pstat, ident, ones8, e_all, GW, GW)
            for g0, s8g in ((0, s8_0), (GW, s8_1)):
                recipg = _recip_bcast(
                    nc, small, pstat, ones_col, ones_row, s8g, GW
                )
                for j in range(GW):
                    b = g0 + j
                    scl = small.tile([P, T], F32, tag="scl")
                    nc.vector.tensor_scalar_mul(
                        scl, e_all[:, b, :], recipg[:, j : j + 1]
                    )
                    tr = ptr.tile([T, P], F32, tag="tr")
                    nc.tensor.transpose(tr, scl, ident)
                    nc.scalar.copy(out=out_t[:, b, :], in_=tr)
                nc.sync.dma_start(
                    out=out_r[:, g0 : g0 + GW, :], in_=out_t[:, g0 : g0 + GW, :]
                )

    nc.compile()
    return nc


def kernel(hidden, encoder_outputs, W_attn, b_attn):
    import ml_dtypes

    hidden = np.asarray(hidden, dtype=np.float32)
    encoder_outputs = np.asarray(encoder_outputs, dtype=np.float32)
    W_attn = np.asarray(W_attn, dtype=np.float32)

    in_maps = []
    for c in range(NCORES):
        bs = slice(c * BL, (c + 1) * BL)
        wx = np.concatenate([W_attn, hidden[0, bs, :].T], axis=1)  # [D, D+BL] f32
        wx_hi = wx.astype(ml_dtypes.bfloat16)
        wx_lo = (wx - wx_hi.astype(np.float32)).astype(ml_dtypes.bfloat16)
        in_maps.append(
            {
                "enc": np.ascontiguousarray(encoder_outputs[:, bs, :]),
                "wxs": np.ascontiguousarray(np.concatenate([wx_hi, wx_lo], axis=1)),
            }
        )

    if "nc" not in _CACHE:
        _CACHE["nc"] = build_kernel()
    nc = _CACHE["nc"]

    res = run_bass_kernel_spmd(nc, in_maps, core_ids=list(range(NCORES)), trace=TRACE)
    if TRACE:
        _CACHE["last_result"] = res
    out = np.concatenate([r["out"] for r in res.results], axis=0)  # [B, S]
    return out[:, None, :]
